# revision 17
# baseline (speedup 1.0000x reference)
"""CoAttentionNetwork Trainium2 kernel — 8-core data parallel over batch.

Takes FULL inputs (B=64), shards batch across 8 NeuronCores (8 batches per
core), runs a Bass/Tile kernel per core, gathers per-core outputs.

v5 design (vs 185us v2 baseline):
  - F = tanh(CWl @ S^T) computed with fp8e4 DoubleRow over the first 256
    of the 384-dim contraction (2x PE throughput) + bf16 for the last 128.
    Validated in numpy: rel_fro ~1.47e-2 vs the 2e-2 gate (all-bf16: 4.9e-3).
    fp8 is used ONLY for these 8 matmuls/batch: v4's full-fp8 pipeline
    (fp8 tanh outputs, fp8 transposes, G DoubleRow) triggered a chip-wide
    ~17% downclock plus slow 1-byte/strided DVE+ACT ops, netting zero.
  - Everything else (F storage, transposes, G, rows, co, fc) stays bf16.
  - Small weights packed into one [128, 254] DMA; wl loads first; all big
    inputs host-prepacked into SBUF layout so DMAs are contiguous.
  - Hs/Hc/logits kept in row form; logits + softmax + fc batched across all
    8 local batches via block-diagonal Whs/Whc; exp fused with row-sums.
"""

import numpy as np

B, N, T, D, K, OUT = 64, 1024, 512, 384, 2, 6
N_CORES = 8
BPC = B // N_CORES  # batches per core
P = 128
NCH = N // P   # 8 n-chunks
TCH = T // P   # 4 t-chunks
DCH = D // P   # 3 d-chunks

# packed-weights column offsets (bf16 [128, WPACK_COLS])
WST_OFF = 0          # Ws^T padded to 32 cols, 3 d-chunks -> 96
WCT_OFF = 96         # Wc^T               -> 96
FCWS_OFF = 192       # fc_w[:, :D]^T, 3 d-chunks x 6 -> 18
FCWC_OFF = 210       # fc_w[:, D:]^T -> 18
WHSBD_OFF = 228      # block-diag Whs [16, 8]
WHCBD_OFF = 236      # block-diag Whc [16, 8]
EYE8_OFF = 244       # eye(8)
EYE2_OFF = 252       # eye(2)
WPACK_COLS = 254

_BUILT = {}


def _build_nc():
    import concourse.bacc as bacc
    import concourse.mybir as mybir
    import concourse.tile as tile

    f32 = mybir.dt.float32
    bf16 = mybir.dt.bfloat16
    fp8 = mybir.dt.float8e4
    AF = mybir.ActivationFunctionType
    DR = mybir.MatmulPerfMode.DoubleRow

    nc = bacc.Bacc(None, target_bir_lowering=False, debug=False)

    # all big inputs are host-prepacked into [P, chunks*len] SBUF layout
    S_d = nc.dram_tensor("S", [BPC, P, NCH * D], bf16, kind="ExternalInput")
    ST_d = nc.dram_tensor("ST", [BPC, P, DCH * N], bf16, kind="ExternalInput")
    ST8_d = nc.dram_tensor("ST8", [BPC, P, 2 * N], fp8, kind="ExternalInput")
    C_d = nc.dram_tensor("C", [BPC, P, TCH * D], bf16, kind="ExternalInput")
    CT_d = nc.dram_tensor("CT", [BPC, P, DCH * T], bf16, kind="ExternalInput")
    Wl_d = nc.dram_tensor("Wl", [P, DCH * D], bf16, kind="ExternalInput")
    wpack_d = nc.dram_tensor("wpack", [P, WPACK_COLS], bf16, kind="ExternalInput")
    ident_d = nc.dram_tensor("ident", [P, P], bf16, kind="ExternalInput")
    fcb_d = nc.dram_tensor("fcb", [BPC, OUT], f32, kind="ExternalInput")
    out_d = nc.dram_tensor("out", [BPC, OUT], f32, kind="ExternalOutput")

    with tile.TileContext(nc) as tc:
        with (
            tc.tile_pool(name="wpool", bufs=1) as wpool,
            tc.tile_pool(name="io", bufs=2) as io,
            tc.tile_pool(name="snat", bufs=BPC) as snat,
            tc.tile_pool(name="cnat", bufs=BPC) as cnat,
            tc.tile_pool(name="work", bufs=2) as work,
            tc.tile_pool(name="fpool", bufs=2) as fpool,
            tc.tile_pool(name="ftpool", bufs=2) as ftpool,
            tc.tile_pool(name="pbig", bufs=3, space="PSUM") as pbig,
            tc.tile_pool(name="prow", bufs=3, space="PSUM") as prow,
            tc.tile_pool(name="psmall", bufs=2, space="PSUM") as psmall,
        ):
            # ---- weights: wl first (gates CWl), then one packed DMA ----
            wl_sb = wpool.tile([P, DCH, D], bf16)
            nc.gpsimd.dma_start(wl_sb[:], Wl_d.rearrange("p (c m) -> p c m", c=DCH))
            wpack_sb = wpool.tile([P, WPACK_COLS], bf16)
            nc.gpsimd.dma_start(wpack_sb[:], wpack_d[:])
            ident_sb = wpool.tile([P, P], bf16)
            nc.gpsimd.dma_start(ident_sb[:], ident_d[:])

            # HAM warmup: the PE clock-gate needs ~3.4us of sustained matmul
            # activity to reach 2.4GHz. Real work can't start until wl+ct
            # land (~12us); these dep-free dummies warm the clock first.
            warm = wpool.tile([P, 512], bf16)
            nc.vector.memset(warm[:], 0.0)
            for _ in range(24):
                pdum = psmall.tile([P, 512], f32, tag="psmall")
                nc.tensor.matmul(pdum[:], warm[:, 0:P], warm[:],
                                 start=True, stop=True)

            def wst(kd):
                return wpack_sb[:, WST_OFF + 32 * kd:WST_OFF + 32 * (kd + 1)]

            def wct(kd):
                return wpack_sb[:, WCT_OFF + 32 * kd:WCT_OFF + 32 * (kd + 1)]

            def fcws(j):
                return wpack_sb[:, FCWS_OFF + OUT * j:FCWS_OFF + OUT * (j + 1)]

            def fcwc(j):
                return wpack_sb[:, FCWC_OFF + OUT * j:FCWC_OFF + OUT * (j + 1)]

            whsbd_sb = wpack_sb[0:2 * BPC, WHSBD_OFF:WHSBD_OFF + BPC]
            whcbd_sb = wpack_sb[0:2 * BPC, WHCBD_OFF:WHCBD_OFF + BPC]
            eye8_sb = wpack_sb[0:BPC, EYE8_OFF:EYE8_OFF + BPC]
            eye2_sb = wpack_sb[0:K, EYE2_OFF:EYE2_OFF + K]

            # persistent row-stacked activations (partition p = 2*b + k)
            hsall = wpool.tile([2 * BPC, N], bf16)
            hcall = wpool.tile([2 * BPC, T], bf16)
            esall = wpool.tile([BPC, N], bf16)
            ecall = wpool.tile([BPC, T], bf16)
            escol = wpool.tile([P, NCH, BPC], bf16)
            eccol = wpool.tile([P, TCH, BPC], bf16)
            out_sb = wpool.tile([BPC, OUT], f32)
            costk = wpool.tile([BPC, 2 * D], bf16)
            ccol_all = wpool.tile([P, 2 * DCH, BPC], bf16)
            fcb_sb = wpool.tile([BPC, OUT], f32)

            s_nats, c_nats, f8s = [], [], []

            def batch_phase1(b):
                # ---- per-batch input DMAs (contiguous, host-prepacked) ----
                ct = io.tile([P, DCH, T], bf16, tag="ct")
                nc.sync.dma_start(ct[:], CT_d[b].rearrange("p (c t) -> p c t", c=DCH))
                st = io.tile([P, DCH, N], bf16, tag="st")
                nc.sync.dma_start(st[:], ST_d[b].rearrange("p (c n) -> p c n", c=DCH))
                st8 = io.tile([P, 2, N], fp8, tag="st8")
                nc.sync.dma_start(st8[:], ST8_d[b].rearrange("p (c n) -> p c n", c=2))
                s_nat = snat.tile([P, NCH, D], bf16, tag="s_nat")
                nc.gpsimd.dma_start(s_nat[:], S_d[b].rearrange("p (c d) -> p c d", c=NCH))
                c_nat = cnat.tile([P, TCH, D], bf16, tag="c_nat")
                nc.gpsimd.dma_start(c_nat[:], C_d[b].rearrange("p (c d) -> p c d", c=TCH))
                s_nats.append(s_nat)
                c_nats.append(c_nat)

                # ---- CWlT [D', t] = Wl^T @ C^T : 3 chunks of [128, 512] ----
                # d-chunks 0,1 stored fp8 (DoubleRow stationary), chunk 2 bf16
                cwlt8 = work.tile([P, 2, T], fp8, tag="cwlt8")
                cwltr = work.tile([P, T], bf16, tag="cwltr")
                for dc in range(DCH):
                    pb = pbig.tile([P, 512], f32, tag="pbig")
                    for kd in range(DCH):
                        nc.tensor.matmul(
                            pb[:], wl_sb[:, kd, dc * P:(dc + 1) * P], ct[:, kd, :],
                            start=(kd == 0), stop=(kd == DCH - 1))
                    if dc == 0:
                        nc.vector.tensor_copy(cwlt8[:, 0, :], pb[:])
                    elif dc == 1:
                        nc.scalar.activation(cwlt8[:, 1, :], pb[:], AF.Copy)
                    else:
                        nc.vector.tensor_copy(cwltr[:], pb[:])

                # ---- F [t, n] = tanh(CWlT^T @ ST): fp8 DR (256) + bf16 (128) ----
                f8 = fpool.tile([P, TCH, N], bf16, tag="f8")
                f8s.append(f8)
                for tcI in range(TCH):
                    pb0 = pbig.tile([P, 512], f32, tag="pbig")
                    pb1 = pbig.tile([P, 512], f32, tag="pbig")
                    tsl = slice(tcI * P, (tcI + 1) * P)
                    nc.tensor.matmul(pb0[:], cwlt8[:, 0:2, tsl], st8[:, 0:2, 0:512],
                                     start=True, stop=False, perf_mode=DR)
                    nc.tensor.matmul(pb1[:], cwlt8[:, 0:2, tsl], st8[:, 0:2, 512:1024],
                                     start=True, stop=False, perf_mode=DR)
                    nc.tensor.matmul(pb0[:], cwltr[:, tsl], st[:, 2, 0:512],
                                     start=False, stop=True)
                    nc.tensor.matmul(pb1[:], cwltr[:, tsl], st[:, 2, 512:1024],
                                     start=False, stop=True)
                    nc.scalar.activation(f8[:, tcI, 0:512], pb0[:], AF.Tanh)
                    nc.scalar.activation(f8[:, tcI, 512:1024], pb1[:], AF.Tanh)

                # ---- WcC row [2, T] (bf16; psum reused by G_c later) ----
                phc = prow.tile([32, T], f32, tag="prow")
                for kd in range(DCH):
                    nc.tensor.matmul(
                        phc[:], wct(kd), ct[:, kd, :],
                        start=(kd == 0), stop=(kd == DCH - 1))
                wcc_row = work.tile([K, T], bf16, tag="wcc_row")
                nc.vector.tensor_copy(wcc_row[:], phc[0:2, :])
                # column form [128, TCH, 16pad] fp8 for G_s DR stationary
                ptc = psmall.tile([P, TCH, K], f32, tag="psmall")
                for tcI in range(TCH):
                    nc.tensor.matmul(
                        ptc[:, tcI, :],
                        wcc_row[:, tcI * P:(tcI + 1) * P], eye2_sb,
                        start=True, stop=True)
                wcc8 = work.tile([P, TCH, K], bf16, tag="wcc8")
                nc.vector.tensor_copy(wcc8[:], ptc[:])

                # ---- WsS rows [2, N] in two half psums (kept for G_s) ----
                phs = []
                for nh in range(2):
                    ph = prow.tile([32, 512], f32, tag="prow")
                    phs.append(ph)
                    for kd in range(DCH):
                        nc.tensor.matmul(
                            ph[:], wst(kd),
                            st[:, kd, nh * 512:(nh + 1) * 512],
                            start=(kd == 0), stop=(kd == DCH - 1))
                wss_row = work.tile([K, N], bf16, tag="wss_row")
                nc.vector.tensor_copy(wss_row[:, 0:512], phs[0][0:2, :])
                nc.vector.tensor_copy(wss_row[:, 512:1024], phs[1][0:2, :])
                # column form [128, NCH, 16pad] fp8 for G_c DR stationary
                ptn = psmall.tile([P, NCH, K], f32, tag="psmall")
                for ncI in range(NCH):
                    nc.tensor.matmul(
                        ptn[:, ncI, :],
                        wss_row[:, ncI * P:(ncI + 1) * P], eye2_sb,
                        start=True, stop=True)
                sws8 = work.tile([P, NCH, K], bf16, tag="sws8")
                nc.vector.tensor_copy(sws8[:], ptn[:])

                # ---- FT [n, t] bf16 via PE transpose of F blocks ----
                ft8 = ftpool.tile([P, NCH, T], bf16, tag="ft8")
                for ncI in range(NCH):
                    pf = pbig.tile([P, T], bf16, tag="pbig")
                    for tcI in range(TCH):
                        nc.tensor.transpose(
                            pf[:, tcI * P:(tcI + 1) * P],
                            f8[:, tcI, ncI * P:(ncI + 1) * P], ident_sb[:])
                    nc.vector.tensor_copy(ft8[:, ncI, :], pf[:])

                # ---- G_s onto WsS psums, tanh -> hs_row ----
                # (engine writes must start at partition 0; the [16, N] stack
                # is assembled by SBUF->SBUF DMA, which has no such limit)
                hs_row = work.tile([K, N], bf16, tag="hs_row")
                for nh in range(2):
                    for tcI in range(TCH):
                        nc.tensor.matmul(
                            phs[nh][0:2, :], wcc8[:, tcI, :],
                            f8[:, tcI, nh * 512:(nh + 1) * 512],
                            start=False, stop=(tcI == TCH - 1),
                            skip_group_check=True)
                    nc.scalar.activation(
                        hs_row[:, nh * 512:(nh + 1) * 512], phs[nh][0:2, :],
                        AF.Tanh)
                nc.gpsimd.dma_start(hsall[2 * b:2 * b + 2, :], hs_row[:])

                # ---- G_c onto WcC psum, tanh -> hc_row ----
                hc_row = work.tile([K, T], bf16, tag="hc_row")
                for ncI in range(NCH):
                    nc.tensor.matmul(
                        phc[0:2, :], sws8[:, ncI, :], ft8[:, ncI, :],
                        start=False, stop=(ncI == NCH - 1),
                        skip_group_check=True)
                nc.scalar.activation(hc_row[:], phc[0:2, :], AF.Tanh)
                nc.gpsimd.dma_start(hcall[2 * b:2 * b + 2, :], hc_row[:])

            for b in range(BPC):
                batch_phase1(b)

            # bridge: PE idles ~2.3us here waiting for the hs/hc row DMAs;
            # dep on batch 7's f8 anchors these after its G work, keeping the
            # clock warm into phase 2.
            for _ in range(12):
                pdum = psmall.tile([P, 512], f32, tag="psmall")
                nc.tensor.matmul(pdum[:], f8s[-1][:, 0, 0:P], f8s[-1][:, 1, 0:512],
                                 start=True, stop=True)

            # ======== phase 2: batched tail across all 8 batches ========
            nc.gpsimd.dma_start(fcb_sb[:], fcb_d[:])
            # logits [8, N]/[8, T] via block-diagonal Whs/Whc
            pls0 = prow.tile([BPC, 512], f32, tag="prow")
            nc.tensor.matmul(pls0[:], whsbd_sb, hsall[:, 0:512],
                             start=True, stop=True)
            pls1 = prow.tile([BPC, 512], f32, tag="prow")
            nc.tensor.matmul(pls1[:], whsbd_sb, hsall[:, 512:1024],
                             start=True, stop=True)
            plc = prow.tile([BPC, 512], f32, tag="prow")
            nc.tensor.matmul(plc[:], whcbd_sb, hcall[:],
                             start=True, stop=True)

            # exp fused with row sums
            acc0 = work.tile([BPC, 1], f32, tag="acc0")
            acc1 = work.tile([BPC, 1], f32, tag="acc1")
            accc = work.tile([BPC, 1], f32, tag="accc")
            nc.scalar.activation(esall[:, 0:512], pls0[:], AF.Exp, accum_out=acc0[:])
            nc.scalar.activation(esall[:, 512:1024], pls1[:], AF.Exp, accum_out=acc1[:])
            nc.scalar.activation(ecall[:], plc[:], AF.Exp, accum_out=accc[:])
            ssum = work.tile([BPC, 1], f32, tag="ssum")
            nc.vector.tensor_add(ssum[:], acc0[:], acc1[:])
            rinv_s = work.tile([BPC, 1], f32, tag="rinv_s")
            nc.vector.reciprocal(rinv_s[:], ssum[:])
            rinv_c = work.tile([BPC, 1], f32, tag="rinv_c")
            nc.vector.reciprocal(rinv_c[:], accc[:])

            # normalize in row form (scalar per partition = per batch)
            nc.vector.tensor_scalar_mul(esall[:, 0:512], esall[:, 0:512],
                                        rinv_s[:])
            nc.vector.tensor_scalar_mul(esall[:, 512:1024], esall[:, 512:1024],
                                        rinv_s[:])
            nc.vector.tensor_scalar_mul(ecall[:], ecall[:], rinv_c[:])

            # es/ec -> column form [128, chunk, batch]
            for ncI in range(NCH):
                pt = psmall.tile([P, BPC], f32, tag="psmall")
                nc.tensor.matmul(pt[:], esall[:, ncI * P:(ncI + 1) * P],
                                 eye8_sb, start=True, stop=True)
                nc.vector.tensor_copy(escol[:, ncI, :], pt[:])
            for tcI in range(TCH):
                pt = psmall.tile([P, BPC], f32, tag="psmall")
                nc.tensor.matmul(pt[:], ecall[:, tcI * P:(tcI + 1) * P],
                                 eye8_sb, start=True, stop=True)
                nc.vector.tensor_copy(eccol[:, tcI, :], pt[:])

            # per-batch co rows (at partition 0), DMA-stacked into costk
            for b in range(BPC):
                pcs = psmall.tile([1, D], f32, tag="psmall")
                for ncI in range(NCH):
                    nc.tensor.matmul(
                        pcs[:], escol[:, ncI, b:b + 1], s_nats[b][:, ncI, :],
                        start=(ncI == 0), stop=(ncI == NCH - 1))
                pcc = psmall.tile([1, D], f32, tag="psmall")
                for tcI in range(TCH):
                    nc.tensor.matmul(
                        pcc[:], eccol[:, tcI, b:b + 1], c_nats[b][:, tcI, :],
                        start=(tcI == 0), stop=(tcI == TCH - 1))
                co_row = work.tile([1, 2 * D], bf16, tag="co_row")
                nc.vector.tensor_copy(co_row[:, 0:D], pcs[:])
                nc.vector.tensor_copy(co_row[:, D:2 * D], pcc[:])
                nc.sync.dma_start(costk[b:b + 1, :], co_row[:])

            # bridge: PE idles ~2.7us waiting on the costk row DMAs
            for _ in range(12):
                pdum = psmall.tile([BPC, 512], f32, tag="psmall")
                nc.tensor.matmul(pdum[:], eccol[:, 0, :], warm[:],
                                 start=True, stop=True)

            # batched fc: transpose costk chunks, 6 accumulating matmuls
            for j in range(2 * DCH):
                pt = psmall.tile([P, BPC], f32, tag="psmall")
                nc.tensor.matmul(pt[:], costk[:, j * P:(j + 1) * P],
                                 eye8_sb, start=True, stop=True)
                nc.vector.tensor_copy(ccol_all[:, j, :], pt[:])
            pout = psmall.tile([BPC, OUT], f32, tag="psmall")
            for j in range(DCH):
                nc.tensor.matmul(pout[:], ccol_all[:, j, :], fcws(j),
                                 start=(j == 0), stop=False)
            for j in range(DCH):
                nc.tensor.matmul(pout[:], ccol_all[:, DCH + j, :],
                                 fcwc(j),
                                 start=False, stop=(j == DCH - 1))
            nc.vector.tensor_add(out_sb[:], pout[:], fcb_sb[:])
            nc.sync.dma_start(out_d[:], out_sb[:])

    nc.compile()
    return nc


def _get_nc():
    if "nc" not in _BUILT:
        _BUILT["nc"] = _build_nc()
    return _BUILT["nc"]


def _to_sbuf_layout(x, p=P):
    """[rows, cols] -> [P, (rows//P)*cols]: row r=c*P+p lands at [p, c*cols:...]."""
    rows, cols = x.shape
    c = rows // p
    return np.ascontiguousarray(
        x.reshape(c, p, cols).transpose(1, 0, 2).reshape(p, c * cols))


def _prep_in_maps(S, C, Wl, Ws, Wc, Whs, Whc, fc_w, fc_b):
    import ml_dtypes

    bf = ml_dtypes.bfloat16
    e4 = ml_dtypes.float8_e4m3
    S = np.ascontiguousarray(np.asarray(S, dtype=np.float32))
    C = np.ascontiguousarray(np.asarray(C, dtype=np.float32))
    Wl = np.asarray(Wl, dtype=np.float32)
    Ws = np.asarray(Ws, dtype=np.float32)
    Wc = np.asarray(Wc, dtype=np.float32)
    Whs = np.asarray(Whs, dtype=np.float32)
    Whc = np.asarray(Whc, dtype=np.float32)
    fc_w = np.asarray(fc_w, dtype=np.float32)
    fc_b = np.asarray(fc_b, dtype=np.float32)

    wpack = np.zeros((P, WPACK_COLS), dtype=np.float32)
    WsT = np.pad(Ws.T, ((0, 0), (0, 32 - K)))  # [D, 32]
    WcT = np.pad(Wc.T, ((0, 0), (0, 32 - K)))
    fcwS = fc_w[:, :D].T  # [D, OUT]
    fcwC = fc_w[:, D:].T
    for c in range(DCH):
        wpack[:, WST_OFF + 32 * c:WST_OFF + 32 * (c + 1)] = WsT[c * P:(c + 1) * P]
        wpack[:, WCT_OFF + 32 * c:WCT_OFF + 32 * (c + 1)] = WcT[c * P:(c + 1) * P]
        wpack[:, FCWS_OFF + OUT * c:FCWS_OFF + OUT * (c + 1)] = fcwS[c * P:(c + 1) * P]
        wpack[:, FCWC_OFF + OUT * c:FCWC_OFF + OUT * (c + 1)] = fcwC[c * P:(c + 1) * P]
    for b in range(BPC):
        wpack[2 * b:2 * b + 2, WHSBD_OFF + b] = Whs[0, :]
        wpack[2 * b:2 * b + 2, WHCBD_OFF + b] = Whc[0, :]
    wpack[0:BPC, EYE8_OFF:EYE8_OFF + BPC] = np.eye(BPC)
    wpack[0:K, EYE2_OFF:EYE2_OFF + K] = np.eye(K)

    Sbf = S.astype(bf)
    Cbf = C.astype(bf)
    in_common = {
        "Wl": _to_sbuf_layout(Wl.astype(np.float32)).astype(bf),
        "wpack": np.ascontiguousarray(wpack.astype(bf)),
        "ident": np.eye(P, dtype=bf),
        "fcb": np.ascontiguousarray(
            np.broadcast_to(fc_b[None, :], (BPC, OUT)).copy()),
    }
    in_maps = []
    for i in range(N_CORES):
        sl = slice(i * BPC, (i + 1) * BPC)
        Sg, Cg = Sbf[sl], Cbf[sl]
        in_maps.append({
            "S": np.stack([_to_sbuf_layout(Sg[j]) for j in range(BPC)]),
            "ST": np.stack([_to_sbuf_layout(np.ascontiguousarray(Sg[j].T))
                            for j in range(BPC)]),
            "ST8": np.stack([_to_sbuf_layout(
                np.ascontiguousarray(Sg[j].T[:2 * P]).astype(np.float32).astype(e4))
                for j in range(BPC)]),
            "C": np.stack([_to_sbuf_layout(Cg[j]) for j in range(BPC)]),
            "CT": np.stack([_to_sbuf_layout(np.ascontiguousarray(Cg[j].T))
                            for j in range(BPC)]),
            **in_common,
        })
    return in_maps


def kernel(S, C, Wl, Ws, Wc, Whs, Whc, fc_w, fc_b):
    from concourse.bass_utils import run_bass_kernel_spmd

    nc = _get_nc()
    in_maps = _prep_in_maps(S, C, Wl, Ws, Wc, Whs, Whc, fc_w, fc_b)
    _BUILT["last_in_maps"] = in_maps
    res = run_bass_kernel_spmd(nc, in_maps, list(range(N_CORES)))
    return np.concatenate(
        [res.results[i]["out"].reshape(BPC, OUT) for i in range(N_CORES)], axis=0)


def __getattr__(name):
    if name == "_LAST_IN_MAPS":
        return _BUILT["last_in_maps"]
    raise AttributeError(name)


# revision 18
# speedup vs baseline: 1.2228x; 1.2228x over previous
"""CoAttentionNetwork Trainium2 kernel — 8-core data parallel over batch.

Takes FULL inputs (B=64), shards batch across 8 NeuronCores (8 batches per
core), runs a Bass/Tile kernel per core, gathers per-core outputs.

v5 design (vs 185us v2 baseline):
  - F = tanh(CWl @ S^T) computed with fp8e4 DoubleRow over the first 256
    of the 384-dim contraction (2x PE throughput) + bf16 for the last 128.
    Validated in numpy: rel_fro ~1.47e-2 vs the 2e-2 gate (all-bf16: 4.9e-3).
    fp8 is used ONLY for these 8 matmuls/batch: v4's full-fp8 pipeline
    (fp8 tanh outputs, fp8 transposes, G DoubleRow) triggered a chip-wide
    ~17% downclock plus slow 1-byte/strided DVE+ACT ops, netting zero.
  - Everything else (F storage, transposes, G, rows, co, fc) stays bf16.
  - Small weights packed into one [128, 254] DMA; wl loads first; all big
    inputs host-prepacked into SBUF layout so DMAs are contiguous.
  - Hs/Hc/logits kept in row form; logits + softmax + fc batched across all
    8 local batches via block-diagonal Whs/Whc; exp fused with row-sums.
"""

import numpy as np

B, N, T, D, K, OUT = 64, 1024, 512, 384, 2, 6
N_CORES = 8
BPC = B // N_CORES  # batches per core
P = 128
NCH = N // P   # 8 n-chunks
TCH = T // P   # 4 t-chunks
DCH = D // P   # 3 d-chunks

# packed-weights column offsets (bf16 [128, WPACK_COLS])
WST_OFF = 0          # Ws^T padded to 32 cols, 3 d-chunks -> 96
WCT_OFF = 96         # Wc^T               -> 96
FCWS_OFF = 192       # fc_w[:, :D]^T, 3 d-chunks x 6 -> 18
FCWC_OFF = 210       # fc_w[:, D:]^T -> 18
WHSBD_OFF = 228      # block-diag Whs [16, 8]
WHCBD_OFF = 236      # block-diag Whc [16, 8]
EYE8_OFF = 244       # eye(8)
EYE2_OFF = 252       # eye(2)
WPACK_COLS = 254

_BUILT = {}


def _build_nc():
    import concourse.bacc as bacc
    import concourse.mybir as mybir
    import concourse.tile as tile

    f32 = mybir.dt.float32
    bf16 = mybir.dt.bfloat16
    fp8 = mybir.dt.float8e4
    AF = mybir.ActivationFunctionType
    DR = mybir.MatmulPerfMode.DoubleRow

    nc = bacc.Bacc(None, target_bir_lowering=False, debug=False)

    # all big inputs are host-prepacked into [P, chunks*len] SBUF layout
    S_d = nc.dram_tensor("S", [BPC, P, NCH * D], bf16, kind="ExternalInput")
    ST_d = nc.dram_tensor("ST", [BPC, P, DCH * N], bf16, kind="ExternalInput")
    ST8_d = nc.dram_tensor("ST8", [BPC, P, 2 * N], fp8, kind="ExternalInput")
    C_d = nc.dram_tensor("C", [BPC, P, TCH * D], bf16, kind="ExternalInput")
    CT_d = nc.dram_tensor("CT", [BPC, P, DCH * T], bf16, kind="ExternalInput")
    Wl_d = nc.dram_tensor("Wl", [P, DCH * D], bf16, kind="ExternalInput")
    wpack_d = nc.dram_tensor("wpack", [P, WPACK_COLS], bf16, kind="ExternalInput")
    ident_d = nc.dram_tensor("ident", [P, P], bf16, kind="ExternalInput")
    fcb_d = nc.dram_tensor("fcb", [BPC, OUT], f32, kind="ExternalInput")
    out_d = nc.dram_tensor("out", [BPC, OUT], f32, kind="ExternalOutput")

    with tile.TileContext(nc) as tc:
        with (
            tc.tile_pool(name="wpool", bufs=1) as wpool,
            tc.tile_pool(name="io", bufs=2) as io,
            tc.tile_pool(name="snat", bufs=BPC) as snat,
            tc.tile_pool(name="cnat", bufs=BPC) as cnat,
            tc.tile_pool(name="work", bufs=2) as work,
            tc.tile_pool(name="fpool", bufs=2) as fpool,
            tc.tile_pool(name="ftpool", bufs=2) as ftpool,
            tc.tile_pool(name="pbig", bufs=3, space="PSUM") as pbig,
            tc.tile_pool(name="prow", bufs=3, space="PSUM") as prow,
            tc.tile_pool(name="psmall", bufs=2, space="PSUM") as psmall,
        ):
            # ---- weights: wl first (gates CWl), then one packed DMA ----
            wl_sb = wpool.tile([P, DCH, D], bf16)
            nc.gpsimd.dma_start(wl_sb[:], Wl_d.rearrange("p (c m) -> p c m", c=DCH))
            wpack_sb = wpool.tile([P, WPACK_COLS], bf16)
            nc.gpsimd.dma_start(wpack_sb[:], wpack_d[:])
            ident_sb = wpool.tile([P, P], bf16)
            nc.gpsimd.dma_start(ident_sb[:], ident_d[:])

            def wst(kd):
                return wpack_sb[:, WST_OFF + 32 * kd:WST_OFF + 32 * (kd + 1)]

            def wct(kd):
                return wpack_sb[:, WCT_OFF + 32 * kd:WCT_OFF + 32 * (kd + 1)]

            def fcws(j):
                return wpack_sb[:, FCWS_OFF + OUT * j:FCWS_OFF + OUT * (j + 1)]

            def fcwc(j):
                return wpack_sb[:, FCWC_OFF + OUT * j:FCWC_OFF + OUT * (j + 1)]

            whsbd_sb = wpack_sb[0:2 * BPC, WHSBD_OFF:WHSBD_OFF + BPC]
            whcbd_sb = wpack_sb[0:2 * BPC, WHCBD_OFF:WHCBD_OFF + BPC]
            eye8_sb = wpack_sb[0:BPC, EYE8_OFF:EYE8_OFF + BPC]
            eye2_sb = wpack_sb[0:K, EYE2_OFF:EYE2_OFF + K]

            # persistent row-stacked activations (partition p = 2*b + k)
            hsall = wpool.tile([2 * BPC, N], bf16)
            hcall = wpool.tile([2 * BPC, T], bf16)
            esall = wpool.tile([BPC, N], bf16)
            ecall = wpool.tile([BPC, T], bf16)
            escol = wpool.tile([P, NCH, BPC], bf16)
            eccol = wpool.tile([P, TCH, BPC], bf16)
            out_sb = wpool.tile([BPC, OUT], f32)
            costk = wpool.tile([BPC, 2 * D], bf16)
            ccol_all = wpool.tile([P, 2 * DCH, BPC], bf16)
            fcb_sb = wpool.tile([BPC, OUT], f32)

            s_nats, c_nats = [], []

            def batch_phase1(b):
                # ---- per-batch input DMAs (contiguous, host-prepacked) ----
                ct = io.tile([P, DCH, T], bf16, tag="ct")
                nc.sync.dma_start(ct[:], CT_d[b].rearrange("p (c t) -> p c t", c=DCH))
                st = io.tile([P, DCH, N], bf16, tag="st")
                nc.sync.dma_start(st[:], ST_d[b].rearrange("p (c n) -> p c n", c=DCH))
                st8 = io.tile([P, 2, N], fp8, tag="st8")
                nc.sync.dma_start(st8[:], ST8_d[b].rearrange("p (c n) -> p c n", c=2))
                s_nat = snat.tile([P, NCH, D], bf16, tag="s_nat")
                nc.gpsimd.dma_start(s_nat[:], S_d[b].rearrange("p (c d) -> p c d", c=NCH))
                c_nat = cnat.tile([P, TCH, D], bf16, tag="c_nat")
                nc.gpsimd.dma_start(c_nat[:], C_d[b].rearrange("p (c d) -> p c d", c=TCH))
                s_nats.append(s_nat)
                c_nats.append(c_nat)

                # ---- CWlT [D', t] = Wl^T @ C^T : 3 chunks of [128, 512] ----
                # d-chunks 0,1 stored fp8 (DoubleRow stationary), chunk 2 bf16
                cwlt8 = work.tile([P, 2, T], fp8, tag="cwlt8")
                cwltr = work.tile([P, T], bf16, tag="cwltr")
                for dc in range(DCH):
                    pb = pbig.tile([P, 512], f32, tag="pbig")
                    for kd in range(DCH):
                        nc.tensor.matmul(
                            pb[:], wl_sb[:, kd, dc * P:(dc + 1) * P], ct[:, kd, :],
                            start=(kd == 0), stop=(kd == DCH - 1))
                    if dc == 0:
                        nc.vector.tensor_copy(cwlt8[:, 0, :], pb[:])
                    elif dc == 1:
                        nc.scalar.activation(cwlt8[:, 1, :], pb[:], AF.Copy)
                    else:
                        nc.vector.tensor_copy(cwltr[:], pb[:])

                # ---- F [t, n] = tanh(CWlT^T @ ST): fp8 DR (256) + bf16 (128) ----
                f8 = fpool.tile([P, TCH, N], bf16, tag="f8")
                for tcI in range(TCH):
                    pb0 = pbig.tile([P, 512], f32, tag="pbig")
                    pb1 = pbig.tile([P, 512], f32, tag="pbig")
                    tsl = slice(tcI * P, (tcI + 1) * P)
                    nc.tensor.matmul(pb0[:], cwlt8[:, 0:2, tsl], st8[:, 0:2, 0:512],
                                     start=True, stop=False, perf_mode=DR)
                    nc.tensor.matmul(pb1[:], cwlt8[:, 0:2, tsl], st8[:, 0:2, 512:1024],
                                     start=True, stop=False, perf_mode=DR)
                    nc.tensor.matmul(pb0[:], cwltr[:, tsl], st[:, 2, 0:512],
                                     start=False, stop=True)
                    nc.tensor.matmul(pb1[:], cwltr[:, tsl], st[:, 2, 512:1024],
                                     start=False, stop=True)
                    nc.scalar.activation(f8[:, tcI, 0:512], pb0[:], AF.Tanh)
                    nc.scalar.activation(f8[:, tcI, 512:1024], pb1[:], AF.Tanh)

                # ---- WcC row [2, T] (bf16; psum reused by G_c later) ----
                phc = prow.tile([32, T], f32, tag="prow")
                for kd in range(DCH):
                    nc.tensor.matmul(
                        phc[:], wct(kd), ct[:, kd, :],
                        start=(kd == 0), stop=(kd == DCH - 1))
                wcc_row = work.tile([K, T], bf16, tag="wcc_row")
                nc.vector.tensor_copy(wcc_row[:], phc[0:2, :])
                # column form [128, TCH, 16pad] fp8 for G_s DR stationary
                ptc = psmall.tile([P, TCH, K], f32, tag="psmall")
                for tcI in range(TCH):
                    nc.tensor.matmul(
                        ptc[:, tcI, :],
                        wcc_row[:, tcI * P:(tcI + 1) * P], eye2_sb,
                        start=True, stop=True)
                wcc8 = work.tile([P, TCH, K], bf16, tag="wcc8")
                nc.vector.tensor_copy(wcc8[:], ptc[:])

                # ---- WsS rows [2, N] in two half psums (kept for G_s) ----
                phs = []
                for nh in range(2):
                    ph = prow.tile([32, 512], f32, tag="prow")
                    phs.append(ph)
                    for kd in range(DCH):
                        nc.tensor.matmul(
                            ph[:], wst(kd),
                            st[:, kd, nh * 512:(nh + 1) * 512],
                            start=(kd == 0), stop=(kd == DCH - 1))
                wss_row = work.tile([K, N], bf16, tag="wss_row")
                nc.vector.tensor_copy(wss_row[:, 0:512], phs[0][0:2, :])
                nc.vector.tensor_copy(wss_row[:, 512:1024], phs[1][0:2, :])
                # column form [128, NCH, 16pad] fp8 for G_c DR stationary
                ptn = psmall.tile([P, NCH, K], f32, tag="psmall")
                for ncI in range(NCH):
                    nc.tensor.matmul(
                        ptn[:, ncI, :],
                        wss_row[:, ncI * P:(ncI + 1) * P], eye2_sb,
                        start=True, stop=True)
                sws8 = work.tile([P, NCH, K], bf16, tag="sws8")
                nc.vector.tensor_copy(sws8[:], ptn[:])

                # ---- FT [n, t] bf16 via PE transpose of F blocks ----
                ft8 = ftpool.tile([P, NCH, T], bf16, tag="ft8")
                for ncI in range(NCH):
                    pf = pbig.tile([P, T], bf16, tag="pbig")
                    for tcI in range(TCH):
                        nc.tensor.transpose(
                            pf[:, tcI * P:(tcI + 1) * P],
                            f8[:, tcI, ncI * P:(ncI + 1) * P], ident_sb[:])
                    nc.vector.tensor_copy(ft8[:, ncI, :], pf[:])

                # ---- G_s onto WsS psums, tanh -> hs_row ----
                # (engine writes must start at partition 0; the [16, N] stack
                # is assembled by SBUF->SBUF DMA, which has no such limit)
                hs_row = work.tile([K, N], bf16, tag="hs_row")
                for nh in range(2):
                    for tcI in range(TCH):
                        nc.tensor.matmul(
                            phs[nh][0:2, :], wcc8[:, tcI, :],
                            f8[:, tcI, nh * 512:(nh + 1) * 512],
                            start=False, stop=(tcI == TCH - 1),
                            skip_group_check=True)
                    nc.scalar.activation(
                        hs_row[:, nh * 512:(nh + 1) * 512], phs[nh][0:2, :],
                        AF.Tanh)
                nc.gpsimd.dma_start(hsall[2 * b:2 * b + 2, :], hs_row[:])

                # ---- G_c onto WcC psum, tanh -> hc_row ----
                hc_row = work.tile([K, T], bf16, tag="hc_row")
                for ncI in range(NCH):
                    nc.tensor.matmul(
                        phc[0:2, :], sws8[:, ncI, :], ft8[:, ncI, :],
                        start=False, stop=(ncI == NCH - 1),
                        skip_group_check=True)
                nc.scalar.activation(hc_row[:], phc[0:2, :], AF.Tanh)
                nc.gpsimd.dma_start(hcall[2 * b:2 * b + 2, :], hc_row[:])

            for b in range(BPC):
                batch_phase1(b)

            # ======== phase 2: batched tail across all 8 batches ========
            nc.gpsimd.dma_start(fcb_sb[:], fcb_d[:])
            # logits [8, N]/[8, T] via block-diagonal Whs/Whc
            pls0 = prow.tile([BPC, 512], f32, tag="prow")
            nc.tensor.matmul(pls0[:], whsbd_sb, hsall[:, 0:512],
                             start=True, stop=True)
            pls1 = prow.tile([BPC, 512], f32, tag="prow")
            nc.tensor.matmul(pls1[:], whsbd_sb, hsall[:, 512:1024],
                             start=True, stop=True)
            plc = prow.tile([BPC, 512], f32, tag="prow")
            nc.tensor.matmul(plc[:], whcbd_sb, hcall[:],
                             start=True, stop=True)

            # exp fused with row sums
            acc0 = work.tile([BPC, 1], f32, tag="acc0")
            acc1 = work.tile([BPC, 1], f32, tag="acc1")
            accc = work.tile([BPC, 1], f32, tag="accc")
            nc.scalar.activation(esall[:, 0:512], pls0[:], AF.Exp, accum_out=acc0[:])
            nc.scalar.activation(esall[:, 512:1024], pls1[:], AF.Exp, accum_out=acc1[:])
            nc.scalar.activation(ecall[:], plc[:], AF.Exp, accum_out=accc[:])
            ssum = work.tile([BPC, 1], f32, tag="ssum")
            nc.vector.tensor_add(ssum[:], acc0[:], acc1[:])
            rinv_s = work.tile([BPC, 1], f32, tag="rinv_s")
            nc.vector.reciprocal(rinv_s[:], ssum[:])
            rinv_c = work.tile([BPC, 1], f32, tag="rinv_c")
            nc.vector.reciprocal(rinv_c[:], accc[:])

            # normalize in row form (scalar per partition = per batch)
            nc.vector.tensor_scalar_mul(esall[:, 0:512], esall[:, 0:512],
                                        rinv_s[:])
            nc.vector.tensor_scalar_mul(esall[:, 512:1024], esall[:, 512:1024],
                                        rinv_s[:])
            nc.vector.tensor_scalar_mul(ecall[:], ecall[:], rinv_c[:])

            # es/ec -> column form [128, chunk, batch]
            for ncI in range(NCH):
                pt = psmall.tile([P, BPC], f32, tag="psmall")
                nc.tensor.matmul(pt[:], esall[:, ncI * P:(ncI + 1) * P],
                                 eye8_sb, start=True, stop=True)
                nc.vector.tensor_copy(escol[:, ncI, :], pt[:])
            for tcI in range(TCH):
                pt = psmall.tile([P, BPC], f32, tag="psmall")
                nc.tensor.matmul(pt[:], ecall[:, tcI * P:(tcI + 1) * P],
                                 eye8_sb, start=True, stop=True)
                nc.vector.tensor_copy(eccol[:, tcI, :], pt[:])

            # per-batch co rows (at partition 0), DMA-stacked into costk
            for b in range(BPC):
                pcs = psmall.tile([1, D], f32, tag="psmall")
                for ncI in range(NCH):
                    nc.tensor.matmul(
                        pcs[:], escol[:, ncI, b:b + 1], s_nats[b][:, ncI, :],
                        start=(ncI == 0), stop=(ncI == NCH - 1))
                pcc = psmall.tile([1, D], f32, tag="psmall")
                for tcI in range(TCH):
                    nc.tensor.matmul(
                        pcc[:], eccol[:, tcI, b:b + 1], c_nats[b][:, tcI, :],
                        start=(tcI == 0), stop=(tcI == TCH - 1))
                co_row = work.tile([1, 2 * D], bf16, tag="co_row")
                nc.vector.tensor_copy(co_row[:, 0:D], pcs[:])
                nc.vector.tensor_copy(co_row[:, D:2 * D], pcc[:])
                nc.sync.dma_start(costk[b:b + 1, :], co_row[:])

            # batched fc: transpose costk chunks, 6 accumulating matmuls
            for j in range(2 * DCH):
                pt = psmall.tile([P, BPC], f32, tag="psmall")
                nc.tensor.matmul(pt[:], costk[:, j * P:(j + 1) * P],
                                 eye8_sb, start=True, stop=True)
                nc.vector.tensor_copy(ccol_all[:, j, :], pt[:])
            pout = psmall.tile([BPC, OUT], f32, tag="psmall")
            for j in range(DCH):
                nc.tensor.matmul(pout[:], ccol_all[:, j, :], fcws(j),
                                 start=(j == 0), stop=False)
            for j in range(DCH):
                nc.tensor.matmul(pout[:], ccol_all[:, DCH + j, :],
                                 fcwc(j),
                                 start=False, stop=(j == DCH - 1))
            nc.vector.tensor_add(out_sb[:], pout[:], fcb_sb[:])
            nc.sync.dma_start(out_d[:], out_sb[:])

    nc.compile()
    return nc


def _get_nc():
    if "nc" not in _BUILT:
        _BUILT["nc"] = _build_nc()
    return _BUILT["nc"]


def _to_sbuf_layout(x, p=P):
    """[rows, cols] -> [P, (rows//P)*cols]: row r=c*P+p lands at [p, c*cols:...]."""
    rows, cols = x.shape
    c = rows // p
    return np.ascontiguousarray(
        x.reshape(c, p, cols).transpose(1, 0, 2).reshape(p, c * cols))


def _prep_in_maps(S, C, Wl, Ws, Wc, Whs, Whc, fc_w, fc_b):
    import ml_dtypes

    bf = ml_dtypes.bfloat16
    e4 = ml_dtypes.float8_e4m3
    S = np.ascontiguousarray(np.asarray(S, dtype=np.float32))
    C = np.ascontiguousarray(np.asarray(C, dtype=np.float32))
    Wl = np.asarray(Wl, dtype=np.float32)
    Ws = np.asarray(Ws, dtype=np.float32)
    Wc = np.asarray(Wc, dtype=np.float32)
    Whs = np.asarray(Whs, dtype=np.float32)
    Whc = np.asarray(Whc, dtype=np.float32)
    fc_w = np.asarray(fc_w, dtype=np.float32)
    fc_b = np.asarray(fc_b, dtype=np.float32)

    wpack = np.zeros((P, WPACK_COLS), dtype=np.float32)
    WsT = np.pad(Ws.T, ((0, 0), (0, 32 - K)))  # [D, 32]
    WcT = np.pad(Wc.T, ((0, 0), (0, 32 - K)))
    fcwS = fc_w[:, :D].T  # [D, OUT]
    fcwC = fc_w[:, D:].T
    for c in range(DCH):
        wpack[:, WST_OFF + 32 * c:WST_OFF + 32 * (c + 1)] = WsT[c * P:(c + 1) * P]
        wpack[:, WCT_OFF + 32 * c:WCT_OFF + 32 * (c + 1)] = WcT[c * P:(c + 1) * P]
        wpack[:, FCWS_OFF + OUT * c:FCWS_OFF + OUT * (c + 1)] = fcwS[c * P:(c + 1) * P]
        wpack[:, FCWC_OFF + OUT * c:FCWC_OFF + OUT * (c + 1)] = fcwC[c * P:(c + 1) * P]
    for b in range(BPC):
        wpack[2 * b:2 * b + 2, WHSBD_OFF + b] = Whs[0, :]
        wpack[2 * b:2 * b + 2, WHCBD_OFF + b] = Whc[0, :]
    wpack[0:BPC, EYE8_OFF:EYE8_OFF + BPC] = np.eye(BPC)
    wpack[0:K, EYE2_OFF:EYE2_OFF + K] = np.eye(K)

    Sbf = S.astype(bf)
    Cbf = C.astype(bf)
    in_common = {
        "Wl": _to_sbuf_layout(Wl.astype(np.float32)).astype(bf),
        "wpack": np.ascontiguousarray(wpack.astype(bf)),
        "ident": np.eye(P, dtype=bf),
        "fcb": np.ascontiguousarray(
            np.broadcast_to(fc_b[None, :], (BPC, OUT)).copy()),
    }
    in_maps = []
    for i in range(N_CORES):
        sl = slice(i * BPC, (i + 1) * BPC)
        Sg, Cg = Sbf[sl], Cbf[sl]
        in_maps.append({
            "S": np.stack([_to_sbuf_layout(Sg[j]) for j in range(BPC)]),
            "ST": np.stack([_to_sbuf_layout(np.ascontiguousarray(Sg[j].T))
                            for j in range(BPC)]),
            "ST8": np.stack([_to_sbuf_layout(
                np.ascontiguousarray(Sg[j].T[:2 * P]).astype(np.float32).astype(e4))
                for j in range(BPC)]),
            "C": np.stack([_to_sbuf_layout(Cg[j]) for j in range(BPC)]),
            "CT": np.stack([_to_sbuf_layout(np.ascontiguousarray(Cg[j].T))
                            for j in range(BPC)]),
            **in_common,
        })
    return in_maps


def kernel(S, C, Wl, Ws, Wc, Whs, Whc, fc_w, fc_b):
    from concourse.bass_utils import run_bass_kernel_spmd

    nc = _get_nc()
    in_maps = _prep_in_maps(S, C, Wl, Ws, Wc, Whs, Whc, fc_w, fc_b)
    _BUILT["last_in_maps"] = in_maps
    res = run_bass_kernel_spmd(nc, in_maps, list(range(N_CORES)))
    return np.concatenate(
        [res.results[i]["out"].reshape(BPC, OUT) for i in range(N_CORES)], axis=0)


def __getattr__(name):
    if name == "_LAST_IN_MAPS":
        return _BUILT["last_in_maps"]
    raise AttributeError(name)
